# revision 4
# baseline (speedup 1.0000x reference)
"""Trainium2 Bass kernel for MMoE (3 tasks, 16 experts, top-4 gating).

Strategy: data-parallel over the batch. Each of the 8 NeuronCores gets
B/8 = 512 tokens and a full (bf16) replica of the expert weights, computes
gating + all 16 expert MLPs + the log-sum-exp combine for its shard, and
writes its [3, 512, 1024] slice. No collectives. Gating logits are computed
in fp32 so top-4 selection matches the reference; expert matmuls run in
bf16 with fp32 PSUM accumulation.

Per-core layout notes:
 - fc1 runs weight-stationary (lhsT = w1^T chunks) so h comes out transposed
   [j, b] — exactly the lhsT layout fc2 needs (contraction over j), avoiding
   any transposes.
 - exp(out) on ScalarE; combine[t] += gate[t,b,e] * exp(out) as a single
   fused scalar_tensor_tensor MAC on VectorE with the gate as a
   per-partition scalar.
 - fc biases are applied for generality: fc1_b via the Relu activation's
   per-partition bias, fc2_b via a K=1 ones-row matmul into PSUM.
"""
import numpy as np
import ml_dtypes

import concourse.mybir as mybir
import concourse.tile as tile
from concourse import bacc
from concourse.bass_utils import run_bass_kernel_spmd

F32 = mybir.dt.float32
BF16 = mybir.dt.bfloat16
AF = mybir.ActivationFunctionType
ALU = mybir.AluOpType
AX = mybir.AxisListType
BF = ml_dtypes.bfloat16

T, B, IN, HID, OUT, E, TOPK = 3, 4096, 1024, 2048, 1024, 16, 4
NCORES = 8
P = 128


class MMoEKernel:
    def __init__(self, bsh=B // NCORES, cin=IN, hid=HID, cout=OUT, ne=E, nt=T):
        self.bsh, self.cin, self.hid, self.cout, self.ne, self.nt = (
            bsh, cin, hid, cout, ne, nt)
        self.nbt = bsh // P
        self.nic = cin // P
        self.njt = hid // P
        self.noh = max(cout // 512, 1)
        self.osz = min(cout, 512)
        self.nq = min(4, self.njt)          # fc1 weight stream granularity
        self.jq = self.njt // self.nq       # j-tiles per fc1 quarter
        self.jh = self.njt // 2             # j-chunks per fc2 half
        self.ng = nt * ne
        self.nc = None

    # ---------------- device graph ----------------
    def build(self):
        bsh, cin, hid, cout, ne, nt = (
            self.bsh, self.cin, self.hid, self.cout, self.ne, self.nt)
        nbt, nic, njt, noh, osz = self.nbt, self.nic, self.njt, self.noh, self.osz
        nq, jq, jh, ng = self.nq, self.jq, self.jh, self.ng

        nc = bacc.Bacc(None, target_bir_lowering=False, debug=False)
        xt = nc.declare_dram_parameter("xt", [P, nic, bsh], F32, isOutput=False)
        wgt = nc.declare_dram_parameter("wgt", [P, nic, ng], F32, isOutput=False)
        w1t = nc.declare_dram_parameter(
            "w1t", [ne, nq, P, nic, hid // nq], BF16, isOutput=False)
        w2t = nc.declare_dram_parameter(
            "w2t", [ne, 2, P, jh, cout], BF16, isOutput=False)
        b1t = nc.declare_dram_parameter("b1t", [P, ne * njt], F32, isOutput=False)
        b2 = nc.declare_dram_parameter("b2", [ne, cout], BF16, isOutput=False)
        out_ext = nc.declare_dram_parameter(
            "out", [nt, bsh, cout], F32, isOutput=True)

        with tile.TileContext(nc) as tc:
            import contextlib
            with contextlib.ExitStack() as ctx:
                const = ctx.enter_context(tc.tile_pool(name="const", bufs=1))
                xf_p = ctx.enter_context(tc.tile_pool(name="xf", bufs=1))
                xb_p = ctx.enter_context(tc.tile_pool(name="xb", bufs=1))
                gat_p = ctx.enter_context(tc.tile_pool(name="gat", bufs=1))
                top_p = ctx.enter_context(tc.tile_pool(name="top", bufs=2))
                w1_p = ctx.enter_context(tc.tile_pool(name="w1", bufs=2))
                w2_p = ctx.enter_context(tc.tile_pool(name="w2", bufs=2))
                b2_p = ctx.enter_context(tc.tile_pool(name="b2", bufs=2))
                h_p = ctx.enter_context(tc.tile_pool(name="h", bufs=2))
                eg_p = ctx.enter_context(tc.tile_pool(name="eg", bufs=2))
                comb_p = ctx.enter_context(tc.tile_pool(name="comb", bufs=1))
                pg_p = ctx.enter_context(
                    tc.tile_pool(name="pg", bufs=2, space="PSUM"))
                ph_p = ctx.enter_context(
                    tc.tile_pool(name="ph", bufs=2, space="PSUM"))
                po_p = ctx.enter_context(
                    tc.tile_pool(name="po", bufs=2, space="PSUM"))

                # resident inputs
                xf = xf_p.tile([P, nic, bsh], F32)
                nc.sync.dma_start(out=xf[:], in_=xt[:, :, :])
                xbf = xb_p.tile([P, nic, bsh], BF16)
                nc.vector.tensor_copy(xbf[:], xf[:])
                wg_sb = const.tile([P, nic, ng], F32)
                nc.sync.dma_start(out=wg_sb[:], in_=wgt[:, :, :])
                b1sb = const.tile([P, ne * njt], F32)
                nc.sync.dma_start(out=b1sb[:], in_=b1t[:, :])
                ones = const.tile([1, P], BF16)
                nc.vector.memset(ones[:], 1.0)
                gates = gat_p.tile([P, nbt, ng], F32)
                comb = comb_p.tile([P, nt * nbt, cout], F32)

                # ---------------- gating (fp32) ----------------
                for bt in range(nbt):
                    pg = pg_p.tile([P, ng], F32)
                    for ic in range(nic):
                        nc.tensor.matmul(
                            pg[:], lhsT=xf[:, ic, bt * P:(bt + 1) * P],
                            rhs=wg_sb[:, ic, :],
                            start=(ic == 0), stop=(ic == nic - 1))
                    gl = top_p.tile([P, ng], F32, tag="gl")
                    nc.scalar.copy(gl[:], pg[:])
                    for t in range(nt):
                        lg = gl[:, t * ne:(t + 1) * ne]
                        m8 = top_p.tile([P, 8], F32, tag="m8")
                        nc.vector.max(m8[:], lg)
                        negm1 = top_p.tile([P, 1], F32, tag="negm1")
                        nc.vector.tensor_scalar_mul(negm1[:], m8[:, 0:1], -1.0)
                        s = top_p.tile([P, ne], F32, tag="s")
                        nc.scalar.activation(s[:], lg, AF.Exp, bias=negm1[:])
                        ind = top_p.tile([P, ne], F32, tag="ind")
                        nc.vector.tensor_scalar(
                            ind[:], lg, m8[:, TOPK - 1:TOPK], None, op0=ALU.is_ge)
                        gun = top_p.tile([P, ne], F32, tag="gun")
                        nc.vector.tensor_mul(gun[:], s[:], ind[:])
                        z = top_p.tile([P, 1], F32, tag="z")
                        nc.vector.reduce_sum(z[:], gun[:], axis=AX.X)
                        rz = top_p.tile([P, 1], F32, tag="rz")
                        nc.vector.reciprocal(rz[:], z[:])
                        gg = top_p.tile([P, ne], F32, tag="gg")
                        nc.vector.tensor_scalar_mul(gg[:], gun[:], rz[:])
                        keep = top_p.tile([P, ne], F32, tag="keep")
                        nc.vector.tensor_scalar(
                            keep[:], gg[:], 1e-4, None, op0=ALU.is_gt)
                        nc.vector.tensor_mul(
                            gates[:, bt, t * ne:(t + 1) * ne], gg[:], keep[:])

                # ---------------- expert loop ----------------
                for e in range(ne):
                    w2h = []
                    for h in range(2):
                        w2sb = w2_p.tile([P, jh, cout], BF16, tag=f"w2h{h}")
                        nc.sync.dma_start(out=w2sb[:], in_=w2t[e, h, :, :, :])
                        w2h.append(w2sb)
                    b2e = b2_p.tile([1, cout], BF16, tag="b2e")
                    nc.sync.dma_start(out=b2e[:], in_=b2[e:e + 1, :])
                    hT = h_p.tile([P, njt, bsh], BF16, tag="hT")
                    w1sb = None
                    for jt in range(njt):
                        q, jj = divmod(jt, jq)
                        if jj == 0:
                            w1sb = w1_p.tile(
                                [P, nic, hid // nq], BF16, tag="w1sb")
                            nc.sync.dma_start(
                                out=w1sb[:], in_=w1t[e, q, :, :, :])
                        ph = ph_p.tile([P, bsh], F32)
                        for ic in range(nic):
                            nc.tensor.matmul(
                                ph[:], lhsT=w1sb[:, ic, jj * P:(jj + 1) * P],
                                rhs=xbf[:, ic, :],
                                start=(ic == 0), stop=(ic == nic - 1))
                        nc.scalar.activation(
                            hT[:, jt, :], ph[:], AF.Relu,
                            bias=b1sb[:, e * njt + jt: e * njt + jt + 1])
                    for bt in range(nbt):
                        po = po_p.tile([P, cout], F32)
                        for oh in range(noh):
                            nc.tensor.matmul(
                                po[:, oh * osz:(oh + 1) * osz], lhsT=ones[:, :],
                                rhs=b2e[:, oh * osz:(oh + 1) * osz],
                                start=True, stop=False)
                        for jc in range(njt):
                            hh, jj = divmod(jc, jh)
                            for oh in range(noh):
                                nc.tensor.matmul(
                                    po[:, oh * osz:(oh + 1) * osz],
                                    lhsT=hT[:, jc, bt * P:(bt + 1) * P],
                                    rhs=w2h[hh][:, jj, oh * osz:(oh + 1) * osz],
                                    start=False, stop=(jc == njt - 1))
                        eg = eg_p.tile([P, cout], F32)
                        nc.scalar.activation(eg[:], po[:], AF.Exp)
                        for t in range(nt):
                            gcol = gates[:, bt, t * ne + e: t * ne + e + 1]
                            dst = comb[:, t * nbt + bt, :]
                            if e == 0:
                                nc.vector.tensor_scalar_mul(dst, eg[:], gcol)
                            else:
                                nc.vector.scalar_tensor_tensor(
                                    dst, eg[:], gcol, dst,
                                    op0=ALU.mult, op1=ALU.add)

                # ---------------- log + output ----------------
                for t in range(nt):
                    for bt in range(nbt):
                        cslice = comb[:, t * nbt + bt, :]
                        nc.scalar.activation(cslice, cslice, AF.Ln)
                        nc.sync.dma_start(
                            out=out_ext[t, bt * P:(bt + 1) * P, :], in_=cslice)

        nc.compile()
        self.nc = nc
        return nc

    # ---------------- host-side marshalling ----------------
    def marshal_shared(self, w_gate, fc1_w, fc1_b, fc2_w, fc2_b):
        cin, hid, cout, ne, nt = self.cin, self.hid, self.cout, self.ne, self.nt
        nic, njt, nq, jh, ng = self.nic, self.njt, self.nq, self.jh, self.ng
        wgt = np.ascontiguousarray(
            w_gate.transpose(1, 0, 2).reshape(cin, ng)
            .reshape(nic, P, ng).transpose(1, 0, 2)).astype(np.float32)
        w1t = np.empty((ne, nq, P, nic, hid // nq), dtype=BF)
        w2t = np.empty((ne, 2, P, jh, cout), dtype=BF)
        for e in range(ne):
            a = fc1_w[e].T.reshape(nic, P, hid).transpose(1, 0, 2)
            for q in range(nq):
                w1t[e, q] = a[:, :, q * (hid // nq):(q + 1) * (hid // nq)]
            bm = fc2_w[e].T.reshape(njt, P, cout).transpose(1, 0, 2)
            for h in range(2):
                w2t[e, h] = bm[:, h * jh:(h + 1) * jh, :]
        b1t = np.ascontiguousarray(
            fc1_b.reshape(ne, njt, P).transpose(2, 0, 1)
            .reshape(P, ne * njt)).astype(np.float32)
        b2m = np.ascontiguousarray(fc2_b).astype(BF)
        return dict(wgt=wgt, w1t=w1t, w2t=w2t, b1t=b1t, b2=b2m)

    def marshal_x(self, x_shard):
        return np.ascontiguousarray(
            x_shard.T.reshape(self.nic, P, self.bsh).transpose(1, 0, 2)
        ).astype(np.float32)

    def run(self, x, w_gate, fc1_w, fc1_b, fc2_w, fc2_b, ncores=NCORES):
        if self.nc is None:
            self.build()
        shared = self.marshal_shared(w_gate, fc1_w, fc1_b, fc2_w, fc2_b)
        in_maps = []
        for c in range(ncores):
            m = dict(shared)
            m["xt"] = self.marshal_x(x[c * self.bsh:(c + 1) * self.bsh])
            in_maps.append(m)
        res = run_bass_kernel_spmd(self.nc, in_maps, core_ids=list(range(ncores)))
        out = np.concatenate(
            [res.results[c]["out"] for c in range(ncores)], axis=1)
        return np.ascontiguousarray(out.astype(np.float32)), res


_KERNEL = None


def kernel(x, w_gate, fc1_w, fc1_b, fc2_w, fc2_b):
    global _KERNEL
    x = np.asarray(x, dtype=np.float32)
    w_gate = np.asarray(w_gate, dtype=np.float32)
    fc1_w = np.asarray(fc1_w, dtype=np.float32)
    fc1_b = np.asarray(fc1_b, dtype=np.float32)
    fc2_w = np.asarray(fc2_w, dtype=np.float32)
    fc2_b = np.asarray(fc2_b, dtype=np.float32)
    if _KERNEL is None:
        _KERNEL = MMoEKernel()
    out, _ = _KERNEL.run(x, w_gate, fc1_w, fc1_b, fc2_w, fc2_b)
    return out


# revision 16
# speedup vs baseline: 1.2264x; 1.2264x over previous
"""Trainium2 Bass kernel for MMoE (3 tasks, 16 experts, top-4 gating).

Strategy: data-parallel over the batch. Each of the 8 NeuronCores gets
B/8 = 512 tokens and a full (bf16) replica of the expert weights, computes
gating + all 16 expert MLPs + the log-sum-exp combine for its shard, and
writes its [3, 512, 1024] slice. No collectives. Gating logits are computed
in fp32 so top-4 selection matches the reference; expert matmuls run in
bf16 with fp32 PSUM accumulation.

Per-core layout notes:
 - fc1 runs weight-stationary (lhsT = w1^T chunks) so h comes out transposed
   [j, b] — exactly the lhsT layout fc2 needs (contraction over j), avoiding
   any transposes.
 - exp(out) on ScalarE; combine[t] += gate[t,b,e] * exp(out) as a single
   fused scalar_tensor_tensor MAC on VectorE with the gate as a
   per-partition scalar.
 - fc biases are applied for generality: fc1_b via the Relu activation's
   per-partition bias, fc2_b via a K=1 ones-row matmul into PSUM.
"""
import os

import numpy as np
import ml_dtypes

import concourse.mybir as mybir
import concourse.tile as tile
from concourse import bacc
import concourse.bass_utils as _bu
from concourse.bass_utils import run_bass_kernel_spmd

if os.environ.get("MMOE_LDW_OPT", "0") == "1" and not getattr(
        _bu, "_mmoe_ldw_patched", False):
    _orig_run_command = _bu.run_command

    def _run_command_ldw(argv, **kw):
        argv = ["--enable-ldw-opt=true" if a == "--enable-ldw-opt=false" else a
                for a in argv]
        return _orig_run_command(argv, **kw)

    _bu.run_command = _run_command_ldw
    _bu._mmoe_ldw_patched = True

F32 = mybir.dt.float32
BF16 = mybir.dt.bfloat16
AF = mybir.ActivationFunctionType
ALU = mybir.AluOpType
AX = mybir.AxisListType
BF = ml_dtypes.bfloat16

T, B, IN, HID, OUT, E, TOPK = 3, 4096, 1024, 2048, 1024, 16, 4
NCORES = 8
P = 128


class MMoEKernel:
    def __init__(self, bsh=B // NCORES, cin=IN, hid=HID, cout=OUT, ne=E, nt=T,
                 use_b2=True):
        self.bsh, self.cin, self.hid, self.cout, self.ne, self.nt = (
            bsh, cin, hid, cout, ne, nt)
        self.use_b2 = use_b2
        self.nbt = bsh // P
        self.nic = cin // P
        self.njt = hid // P
        self.noh = max(cout // 512, 1)
        self.osz = min(cout, 512)
        self.nq = min(4, self.njt)          # fc1 weight stream granularity
        self.jq = self.njt // self.nq       # j-tiles per fc1 quarter
        self.jh = self.njt // 2             # j-chunks per fc2 half
        self.ng = nt * ne
        self.nc = None

    # ---------------- device graph ----------------
    def build(self):
        bsh, cin, hid, cout, ne, nt = (
            self.bsh, self.cin, self.hid, self.cout, self.ne, self.nt)
        nbt, nic, njt, noh, osz = self.nbt, self.nic, self.njt, self.noh, self.osz
        nq, jq, jh, ng = self.nq, self.jq, self.jh, self.ng

        nc = bacc.Bacc(None, target_bir_lowering=False, debug=False)
        xth = nc.declare_dram_parameter("xth", [P, nic, bsh], BF16, isOutput=False)
        xtl = nc.declare_dram_parameter("xtl", [P, nic, bsh], BF16, isOutput=False)
        wgh = nc.declare_dram_parameter("wgh", [P, nic, ng], BF16, isOutput=False)
        wgl = nc.declare_dram_parameter("wgl", [P, nic, ng], BF16, isOutput=False)
        w1t = nc.declare_dram_parameter(
            "w1t", [ne, nq, P, nic, hid // nq], BF16, isOutput=False)
        w2t = nc.declare_dram_parameter(
            "w2t", [ne, 2, P, jh, cout], BF16, isOutput=False)
        b1t = nc.declare_dram_parameter("b1t", [P, ne * njt], F32, isOutput=False)
        b2 = nc.declare_dram_parameter("b2", [ne, cout], BF16, isOutput=False)
        out_ext = nc.declare_dram_parameter(
            "out", [nt, bsh, cout], F32, isOutput=True)

        with tile.TileContext(nc) as tc:
            import contextlib
            with contextlib.ExitStack() as ctx:
                const = ctx.enter_context(tc.tile_pool(name="const", bufs=1))
                xf_p = ctx.enter_context(tc.tile_pool(name="xf", bufs=1))
                xb_p = ctx.enter_context(tc.tile_pool(name="xb", bufs=1))
                gat_p = ctx.enter_context(tc.tile_pool(name="gat", bufs=1))
                top_p = ctx.enter_context(tc.tile_pool(name="top", bufs=2))
                w1_p = ctx.enter_context(tc.tile_pool(name="w1", bufs=2))
                w2_p = ctx.enter_context(tc.tile_pool(name="w2", bufs=2))
                b2_p = ctx.enter_context(tc.tile_pool(name="b2", bufs=2))
                h_p = ctx.enter_context(tc.tile_pool(name="h", bufs=2))
                eg_p = ctx.enter_context(tc.tile_pool(name="eg", bufs=2))
                comb_p = ctx.enter_context(tc.tile_pool(name="comb", bufs=1))
                pg_p = ctx.enter_context(
                    tc.tile_pool(name="pg", bufs=2, space="PSUM"))
                ph_p = ctx.enter_context(
                    tc.tile_pool(name="ph", bufs=2, space="PSUM"))
                po_p = ctx.enter_context(
                    tc.tile_pool(name="po", bufs=2, space="PSUM"))

                # resident inputs (x in bf16 hi+lo split: hi carries bf16(x),
                # lo the residual, so gating logits reach ~fp32 accuracy with
                # only bf16 matmuls in the PE stream)
                xbf = xb_p.tile([P, nic, bsh], BF16)
                nc.sync.dma_start(out=xbf[:], in_=xth[:, :, :])
                xlo = xf_p.tile([P, nic, bsh], BF16)
                nc.sync.dma_start(out=xlo[:], in_=xtl[:, :, :])
                wg_h = const.tile([P, nic, ng], BF16)
                nc.sync.dma_start(out=wg_h[:], in_=wgh[:, :, :])
                wg_l = const.tile([P, nic, ng], BF16)
                nc.sync.dma_start(out=wg_l[:], in_=wgl[:, :, :])
                b1sb = const.tile([P, ne * njt], F32)
                nc.sync.dma_start(out=b1sb[:], in_=b1t[:, :])
                ones = const.tile([1, P], BF16)
                nc.vector.memset(ones[:], 1.0)
                gates = gat_p.tile([P, nbt, ng], F32)
                comb = comb_p.tile([P, nt * nbt, cout], F32)

                # ---------------- gating (fp32) ----------------
                for bt in range(nbt):
                    pg = pg_p.tile([P, ng], F32)
                    pairs = [(xbf, wg_h), (xbf, wg_l), (xlo, wg_h), (xlo, wg_l)]
                    for pi, (xa, wa) in enumerate(pairs):
                        for ic in range(nic):
                            nc.tensor.matmul(
                                pg[:], lhsT=xa[:, ic, bt * P:(bt + 1) * P],
                                rhs=wa[:, ic, :],
                                start=(pi == 0 and ic == 0),
                                stop=(pi == 3 and ic == nic - 1))
                    gl = top_p.tile([P, ng], F32, tag="gl")
                    nc.scalar.copy(gl[:], pg[:])
                    for t in range(nt):
                        lg = gl[:, t * ne:(t + 1) * ne]
                        m8 = top_p.tile([P, 8], F32, tag="m8")
                        nc.vector.max(m8[:], lg)
                        negm1 = top_p.tile([P, 1], F32, tag="negm1")
                        nc.vector.tensor_scalar_mul(negm1[:], m8[:, 0:1], -1.0)
                        s = top_p.tile([P, ne], F32, tag="s")
                        nc.scalar.activation(s[:], lg, AF.Exp, bias=negm1[:])
                        ind = top_p.tile([P, ne], F32, tag="ind")
                        nc.vector.tensor_scalar(
                            ind[:], lg, m8[:, TOPK - 1:TOPK], None, op0=ALU.is_ge)
                        gun = top_p.tile([P, ne], F32, tag="gun")
                        nc.vector.tensor_mul(gun[:], s[:], ind[:])
                        z = top_p.tile([P, 1], F32, tag="z")
                        nc.vector.reduce_sum(z[:], gun[:], axis=AX.X)
                        rz = top_p.tile([P, 1], F32, tag="rz")
                        nc.vector.reciprocal(rz[:], z[:])
                        gg = top_p.tile([P, ne], F32, tag="gg")
                        nc.vector.tensor_scalar_mul(gg[:], gun[:], rz[:])
                        keep = top_p.tile([P, ne], F32, tag="keep")
                        nc.vector.tensor_scalar(
                            keep[:], gg[:], 1e-4, None, op0=ALU.is_gt)
                        nc.vector.tensor_mul(
                            gates[:, bt, t * ne:(t + 1) * ne], gg[:], keep[:])

                # ---------------- expert loop ----------------
                for e in range(ne):
                    w2h = []
                    for h in range(2):
                        w2sb = w2_p.tile([P, jh, cout], BF16, tag=f"w2h{h}")
                        nc.sync.dma_start(out=w2sb[:], in_=w2t[e, h, :, :, :])
                        w2h.append(w2sb)
                    b2e = b2_p.tile([1, cout], BF16, tag="b2e")
                    nc.sync.dma_start(out=b2e[:], in_=b2[e:e + 1, :])
                    hT = h_p.tile([P, njt, bsh], BF16, tag="hT")
                    w1sb = None
                    for jt in range(njt):
                        q, jj = divmod(jt, jq)
                        if jj == 0:
                            w1sb = w1_p.tile(
                                [P, nic, hid // nq], BF16, tag="w1sb")
                            nc.sync.dma_start(
                                out=w1sb[:], in_=w1t[e, q, :, :, :])
                        ph = ph_p.tile([P, bsh], F32)
                        for ic in range(nic):
                            nc.tensor.matmul(
                                ph[:], lhsT=w1sb[:, ic, jj * P:(jj + 1) * P],
                                rhs=xbf[:, ic, :],
                                start=(ic == 0), stop=(ic == nic - 1))
                        nc.scalar.activation(
                            hT[:, jt, :], ph[:], AF.Relu,
                            bias=b1sb[:, e * njt + jt: e * njt + jt + 1])
                    for bt in range(nbt):
                        po = po_p.tile([P, cout], F32)
                        if self.use_b2:
                            for oh in range(noh):
                                nc.tensor.matmul(
                                    po[:, oh * osz:(oh + 1) * osz],
                                    lhsT=ones[:, :],
                                    rhs=b2e[:, oh * osz:(oh + 1) * osz],
                                    start=True, stop=False)
                        for jc in range(njt):
                            hh, jj = divmod(jc, jh)
                            for oh in range(noh):
                                nc.tensor.matmul(
                                    po[:, oh * osz:(oh + 1) * osz],
                                    lhsT=hT[:, jc, bt * P:(bt + 1) * P],
                                    rhs=w2h[hh][:, jj, oh * osz:(oh + 1) * osz],
                                    start=(jc == 0 and not self.use_b2),
                                    stop=(jc == njt - 1))
                        eg = eg_p.tile([P, cout], F32)
                        nc.scalar.activation(eg[:], po[:], AF.Exp)
                        for t in range(nt):
                            gcol = gates[:, bt, t * ne + e: t * ne + e + 1]
                            dst = comb[:, t * nbt + bt, :]
                            if e == 0:
                                nc.vector.tensor_scalar_mul(dst, eg[:], gcol)
                            else:
                                nc.vector.scalar_tensor_tensor(
                                    dst, eg[:], gcol, dst,
                                    op0=ALU.mult, op1=ALU.add)

                # ---------------- log + output ----------------
                for t in range(nt):
                    for bt in range(nbt):
                        cslice = comb[:, t * nbt + bt, :]
                        nc.scalar.activation(cslice, cslice, AF.Ln)
                        nc.sync.dma_start(
                            out=out_ext[t, bt * P:(bt + 1) * P, :], in_=cslice)

        nc.compile()
        self.nc = nc
        return nc

    # ---------------- host-side marshalling ----------------
    def marshal_shared(self, w_gate, fc1_w, fc1_b, fc2_w, fc2_b):
        cin, hid, cout, ne, nt = self.cin, self.hid, self.cout, self.ne, self.nt
        nic, njt, nq, jh, ng = self.nic, self.njt, self.nq, self.jh, self.ng
        wgt = np.ascontiguousarray(
            w_gate.transpose(1, 0, 2).reshape(cin, ng)
            .reshape(nic, P, ng).transpose(1, 0, 2)).astype(np.float32)
        wgh = wgt.astype(BF)
        wgl = (wgt - wgh.astype(np.float32)).astype(BF)
        w1t = np.empty((ne, nq, P, nic, hid // nq), dtype=BF)
        w2t = np.empty((ne, 2, P, jh, cout), dtype=BF)
        for e in range(ne):
            a = fc1_w[e].T.reshape(nic, P, hid).transpose(1, 0, 2)
            for q in range(nq):
                w1t[e, q] = a[:, :, q * (hid // nq):(q + 1) * (hid // nq)]
            bm = fc2_w[e].T.reshape(njt, P, cout).transpose(1, 0, 2)
            for h in range(2):
                w2t[e, h] = bm[:, h * jh:(h + 1) * jh, :]
        b1t = np.ascontiguousarray(
            fc1_b.reshape(ne, njt, P).transpose(2, 0, 1)
            .reshape(P, ne * njt)).astype(np.float32)
        b2m = np.ascontiguousarray(fc2_b).astype(BF)
        return dict(wgh=wgh, wgl=wgl, w1t=w1t, w2t=w2t, b1t=b1t, b2=b2m)

    def marshal_x(self, x_shard):
        xt = np.ascontiguousarray(
            x_shard.T.reshape(self.nic, P, self.bsh).transpose(1, 0, 2)
        ).astype(np.float32)
        xh = xt.astype(BF)
        xl = (xt - xh.astype(np.float32)).astype(BF)
        return xh, xl

    def run(self, x, w_gate, fc1_w, fc1_b, fc2_w, fc2_b, ncores=NCORES):
        if self.nc is None:
            self.build()
        shared = self.marshal_shared(w_gate, fc1_w, fc1_b, fc2_w, fc2_b)
        in_maps = []
        for c in range(ncores):
            m = dict(shared)
            m["xth"], m["xtl"] = self.marshal_x(
                x[c * self.bsh:(c + 1) * self.bsh])
            in_maps.append(m)
        res = run_bass_kernel_spmd(self.nc, in_maps, core_ids=list(range(ncores)))
        out = np.concatenate(
            [res.results[c]["out"] for c in range(ncores)], axis=1)
        return np.ascontiguousarray(out.astype(np.float32)), res


_KERNEL = None


def kernel(x, w_gate, fc1_w, fc1_b, fc2_w, fc2_b):
    global _KERNEL
    x = np.asarray(x, dtype=np.float32)
    w_gate = np.asarray(w_gate, dtype=np.float32)
    fc1_w = np.asarray(fc1_w, dtype=np.float32)
    fc1_b = np.asarray(fc1_b, dtype=np.float32)
    fc2_w = np.asarray(fc2_w, dtype=np.float32)
    fc2_b = np.asarray(fc2_b, dtype=np.float32)
    if _KERNEL is None:
        _KERNEL = MMoEKernel(use_b2=bool(np.any(fc2_b)))
    out, _ = _KERNEL.run(x, w_gate, fc1_w, fc1_b, fc2_w, fc2_b)
    return out


# revision 18
# speedup vs baseline: 1.2444x; 1.0147x over previous
"""Trainium2 Bass kernel for MMoE (3 tasks, 16 experts, top-4 gating).

Strategy: data-parallel over the batch. Each of the 8 NeuronCores gets
B/8 = 512 tokens and a full (bf16) replica of the expert weights, computes
gating + all 16 expert MLPs + the log-sum-exp combine for its shard, and
writes its [3, 512, 1024] slice. No collectives. Gating logits are computed
in fp32 so top-4 selection matches the reference; expert matmuls run in
bf16 with fp32 PSUM accumulation.

Per-core layout notes:
 - fc1 runs weight-stationary (lhsT = w1^T chunks) so h comes out transposed
   [j, b] — exactly the lhsT layout fc2 needs (contraction over j), avoiding
   any transposes.
 - exp(out) on ScalarE; combine[t] += gate[t,b,e] * exp(out) as a single
   fused scalar_tensor_tensor MAC on VectorE with the gate as a
   per-partition scalar.
 - fc biases are applied for generality: fc1_b via the Relu activation's
   per-partition bias, fc2_b via a K=1 ones-row matmul into PSUM.
"""
import os

import numpy as np
import ml_dtypes

import concourse.mybir as mybir
import concourse.tile as tile
from concourse import bacc
import concourse.bass_utils as _bu
from concourse.bass_utils import run_bass_kernel_spmd

if os.environ.get("MMOE_LDW_OPT", "0") == "1" and not getattr(
        _bu, "_mmoe_ldw_patched", False):
    _orig_run_command = _bu.run_command

    def _run_command_ldw(argv, **kw):
        argv = ["--enable-ldw-opt=true" if a == "--enable-ldw-opt=false" else a
                for a in argv]
        return _orig_run_command(argv, **kw)

    _bu.run_command = _run_command_ldw
    _bu._mmoe_ldw_patched = True

F32 = mybir.dt.float32
BF16 = mybir.dt.bfloat16
AF = mybir.ActivationFunctionType
ALU = mybir.AluOpType
AX = mybir.AxisListType
BF = ml_dtypes.bfloat16

T, B, IN, HID, OUT, E, TOPK = 3, 4096, 1024, 2048, 1024, 16, 4
NCORES = 8
P = 128


class MMoEKernel:
    def __init__(self, bsh=B // NCORES, cin=IN, hid=HID, cout=OUT, ne=E, nt=T,
                 use_b2=True):
        self.bsh, self.cin, self.hid, self.cout, self.ne, self.nt = (
            bsh, cin, hid, cout, ne, nt)
        self.use_b2 = use_b2
        self.nbt = bsh // P
        self.nic = cin // P
        self.njt = hid // P
        self.noh = max(cout // 512, 1)
        self.osz = min(cout, 512)
        self.nq = min(4, self.njt)          # fc1 weight stream granularity
        self.jq = self.njt // self.nq       # j-tiles per fc1 quarter
        self.jh = self.njt // 2             # j-chunks per fc2 half
        self.ng = nt * ne
        self.nc = None

    # ---------------- device graph ----------------
    def build(self):
        bsh, cin, hid, cout, ne, nt = (
            self.bsh, self.cin, self.hid, self.cout, self.ne, self.nt)
        nbt, nic, njt, noh, osz = self.nbt, self.nic, self.njt, self.noh, self.osz
        nq, jq, jh, ng = self.nq, self.jq, self.jh, self.ng

        nc = bacc.Bacc(None, target_bir_lowering=False, debug=False)
        xth = nc.declare_dram_parameter("xth", [P, nic, bsh], BF16, isOutput=False)
        xtl = nc.declare_dram_parameter("xtl", [P, nic, bsh], BF16, isOutput=False)
        wgh = nc.declare_dram_parameter("wgh", [P, nic, ng], BF16, isOutput=False)
        wgl = nc.declare_dram_parameter("wgl", [P, nic, ng], BF16, isOutput=False)
        w1t = nc.declare_dram_parameter(
            "w1t", [ne, nq, P, nic, hid // nq], BF16, isOutput=False)
        w2t = nc.declare_dram_parameter(
            "w2t", [ne, 2, P, jh, cout], BF16, isOutput=False)
        b1t = nc.declare_dram_parameter("b1t", [P, ne * njt], F32, isOutput=False)
        b2 = nc.declare_dram_parameter("b2", [ne, cout], BF16, isOutput=False)
        out_ext = nc.declare_dram_parameter(
            "out", [nt, bsh, cout], F32, isOutput=True)

        with tile.TileContext(nc) as tc:
            import contextlib
            with contextlib.ExitStack() as ctx:
                const = ctx.enter_context(tc.tile_pool(name="const", bufs=1))
                xf_p = ctx.enter_context(tc.tile_pool(name="xf", bufs=1))
                xb_p = ctx.enter_context(tc.tile_pool(name="xb", bufs=1))
                gat_p = ctx.enter_context(tc.tile_pool(name="gat", bufs=1))
                top_p = ctx.enter_context(tc.tile_pool(name="top", bufs=2))
                w1_p = ctx.enter_context(tc.tile_pool(name="w1", bufs=2))
                w2_p = ctx.enter_context(tc.tile_pool(name="w2", bufs=2))
                b2_p = ctx.enter_context(tc.tile_pool(name="b2", bufs=2))
                h_p = ctx.enter_context(tc.tile_pool(name="h", bufs=2))
                eg_p = ctx.enter_context(tc.tile_pool(name="eg", bufs=2))
                comb_p = ctx.enter_context(tc.tile_pool(name="comb", bufs=1))
                pg_p = ctx.enter_context(
                    tc.tile_pool(name="pg", bufs=2, space="PSUM"))
                ph_p = ctx.enter_context(
                    tc.tile_pool(name="ph", bufs=2, space="PSUM"))
                po_p = ctx.enter_context(
                    tc.tile_pool(name="po", bufs=2, space="PSUM"))

                # prefetch expert 0 weights first so fc1 can start the moment
                # gating's tiny matmuls finish
                pre_w2h = []
                for h in range(2):
                    w2sb = w2_p.tile([P, jh, cout], BF16, tag=f"w2h{h}")
                    nc.sync.dma_start(out=w2sb[:], in_=w2t[0, h, :, :, :])
                    pre_w2h.append(w2sb)
                pre_b2e = b2_p.tile([1, cout], BF16, tag="b2e")
                nc.sync.dma_start(out=pre_b2e[:], in_=b2[0:1, :])
                pre_w1sb = w1_p.tile([P, nic, hid // nq], BF16, tag="w1sb")
                nc.sync.dma_start(out=pre_w1sb[:], in_=w1t[0, 0, :, :, :])

                # resident inputs (x in bf16 hi+lo split: hi carries bf16(x),
                # lo the residual, so gating logits reach ~fp32 accuracy with
                # only bf16 matmuls in the PE stream)
                xbf = xb_p.tile([P, nic, bsh], BF16)
                nc.sync.dma_start(out=xbf[:], in_=xth[:, :, :])
                xlo = xf_p.tile([P, nic, bsh], BF16)
                nc.sync.dma_start(out=xlo[:], in_=xtl[:, :, :])
                wg_h = const.tile([P, nic, ng], BF16)
                nc.sync.dma_start(out=wg_h[:], in_=wgh[:, :, :])
                wg_l = const.tile([P, nic, ng], BF16)
                nc.sync.dma_start(out=wg_l[:], in_=wgl[:, :, :])
                b1sb = const.tile([P, ne * njt], F32)
                nc.sync.dma_start(out=b1sb[:], in_=b1t[:, :])
                ones = const.tile([1, P], BF16)
                nc.vector.memset(ones[:], 1.0)
                gates = gat_p.tile([P, nbt, ng], F32)
                comb = comb_p.tile([P, nt * nbt, cout], F32)

                # ---------------- gating (fp32) ----------------
                for bt in range(nbt):
                    pg = pg_p.tile([P, ng], F32)
                    pairs = [(xbf, wg_h), (xbf, wg_l), (xlo, wg_h), (xlo, wg_l)]
                    for pi, (xa, wa) in enumerate(pairs):
                        for ic in range(nic):
                            nc.tensor.matmul(
                                pg[:], lhsT=xa[:, ic, bt * P:(bt + 1) * P],
                                rhs=wa[:, ic, :],
                                start=(pi == 0 and ic == 0),
                                stop=(pi == 3 and ic == nic - 1))
                    gl = top_p.tile([P, ng], F32, tag="gl")
                    nc.scalar.copy(gl[:], pg[:])
                    for t in range(nt):
                        lg = gl[:, t * ne:(t + 1) * ne]
                        m8 = top_p.tile([P, 8], F32, tag="m8")
                        nc.vector.max(m8[:], lg)
                        negm1 = top_p.tile([P, 1], F32, tag="negm1")
                        nc.vector.tensor_scalar_mul(negm1[:], m8[:, 0:1], -1.0)
                        s = top_p.tile([P, ne], F32, tag="s")
                        nc.scalar.activation(s[:], lg, AF.Exp, bias=negm1[:])
                        ind = top_p.tile([P, ne], F32, tag="ind")
                        nc.vector.tensor_scalar(
                            ind[:], lg, m8[:, TOPK - 1:TOPK], None, op0=ALU.is_ge)
                        gun = top_p.tile([P, ne], F32, tag="gun")
                        nc.vector.tensor_mul(gun[:], s[:], ind[:])
                        z = top_p.tile([P, 1], F32, tag="z")
                        nc.vector.reduce_sum(z[:], gun[:], axis=AX.X)
                        rz = top_p.tile([P, 1], F32, tag="rz")
                        nc.vector.reciprocal(rz[:], z[:])
                        gg = top_p.tile([P, ne], F32, tag="gg")
                        nc.vector.tensor_scalar_mul(gg[:], gun[:], rz[:])
                        keep = top_p.tile([P, ne], F32, tag="keep")
                        nc.vector.tensor_scalar(
                            keep[:], gg[:], 1e-4, None, op0=ALU.is_gt)
                        nc.vector.tensor_mul(
                            gates[:, bt, t * ne:(t + 1) * ne], gg[:], keep[:])

                # ---------------- expert loop ----------------
                for e in range(ne):
                    if e == 0:
                        w2h = pre_w2h
                        b2e = pre_b2e
                    else:
                        w2h = []
                        for h in range(2):
                            w2sb = w2_p.tile([P, jh, cout], BF16, tag=f"w2h{h}")
                            nc.sync.dma_start(
                                out=w2sb[:], in_=w2t[e, h, :, :, :])
                            w2h.append(w2sb)
                        b2e = b2_p.tile([1, cout], BF16, tag="b2e")
                        nc.sync.dma_start(out=b2e[:], in_=b2[e:e + 1, :])
                    hT = h_p.tile([P, njt, bsh], BF16, tag="hT")
                    w1sb = None
                    for jt in range(njt):
                        q, jj = divmod(jt, jq)
                        if jj == 0:
                            if e == 0 and q == 0:
                                w1sb = pre_w1sb
                            else:
                                w1sb = w1_p.tile(
                                    [P, nic, hid // nq], BF16, tag="w1sb")
                                nc.sync.dma_start(
                                    out=w1sb[:], in_=w1t[e, q, :, :, :])
                        ph = ph_p.tile([P, bsh], F32)
                        for ic in range(nic):
                            nc.tensor.matmul(
                                ph[:], lhsT=w1sb[:, ic, jj * P:(jj + 1) * P],
                                rhs=xbf[:, ic, :],
                                start=(ic == 0), stop=(ic == nic - 1))
                        nc.scalar.activation(
                            hT[:, jt, :], ph[:], AF.Relu,
                            bias=b1sb[:, e * njt + jt: e * njt + jt + 1])
                    for bt in range(nbt):
                        po = po_p.tile([P, cout], F32)
                        if self.use_b2:
                            for oh in range(noh):
                                nc.tensor.matmul(
                                    po[:, oh * osz:(oh + 1) * osz],
                                    lhsT=ones[:, :],
                                    rhs=b2e[:, oh * osz:(oh + 1) * osz],
                                    start=True, stop=False)
                        for jc in range(njt):
                            hh, jj = divmod(jc, jh)
                            for oh in range(noh):
                                nc.tensor.matmul(
                                    po[:, oh * osz:(oh + 1) * osz],
                                    lhsT=hT[:, jc, bt * P:(bt + 1) * P],
                                    rhs=w2h[hh][:, jj, oh * osz:(oh + 1) * osz],
                                    start=(jc == 0 and not self.use_b2),
                                    stop=(jc == njt - 1))
                        eg = eg_p.tile([P, cout], F32)
                        nc.scalar.activation(eg[:], po[:], AF.Exp)
                        for t in range(nt):
                            gcol = gates[:, bt, t * ne + e: t * ne + e + 1]
                            dst = comb[:, t * nbt + bt, :]
                            if e == 0:
                                nc.vector.tensor_scalar_mul(dst, eg[:], gcol)
                            else:
                                nc.vector.scalar_tensor_tensor(
                                    dst, eg[:], gcol, dst,
                                    op0=ALU.mult, op1=ALU.add)

                # ---------------- log + output ----------------
                for t in range(nt):
                    for bt in range(nbt):
                        cslice = comb[:, t * nbt + bt, :]
                        nc.scalar.activation(cslice, cslice, AF.Ln)
                        nc.sync.dma_start(
                            out=out_ext[t, bt * P:(bt + 1) * P, :], in_=cslice)

        nc.compile()
        self.nc = nc
        return nc

    # ---------------- host-side marshalling ----------------
    def marshal_shared(self, w_gate, fc1_w, fc1_b, fc2_w, fc2_b):
        cin, hid, cout, ne, nt = self.cin, self.hid, self.cout, self.ne, self.nt
        nic, njt, nq, jh, ng = self.nic, self.njt, self.nq, self.jh, self.ng
        wgt = np.ascontiguousarray(
            w_gate.transpose(1, 0, 2).reshape(cin, ng)
            .reshape(nic, P, ng).transpose(1, 0, 2)).astype(np.float32)
        wgh = wgt.astype(BF)
        wgl = (wgt - wgh.astype(np.float32)).astype(BF)
        w1t = np.empty((ne, nq, P, nic, hid // nq), dtype=BF)
        w2t = np.empty((ne, 2, P, jh, cout), dtype=BF)
        for e in range(ne):
            a = fc1_w[e].T.reshape(nic, P, hid).transpose(1, 0, 2)
            for q in range(nq):
                w1t[e, q] = a[:, :, q * (hid // nq):(q + 1) * (hid // nq)]
            bm = fc2_w[e].T.reshape(njt, P, cout).transpose(1, 0, 2)
            for h in range(2):
                w2t[e, h] = bm[:, h * jh:(h + 1) * jh, :]
        b1t = np.ascontiguousarray(
            fc1_b.reshape(ne, njt, P).transpose(2, 0, 1)
            .reshape(P, ne * njt)).astype(np.float32)
        b2m = np.ascontiguousarray(fc2_b).astype(BF)
        return dict(wgh=wgh, wgl=wgl, w1t=w1t, w2t=w2t, b1t=b1t, b2=b2m)

    def marshal_x(self, x_shard):
        xt = np.ascontiguousarray(
            x_shard.T.reshape(self.nic, P, self.bsh).transpose(1, 0, 2)
        ).astype(np.float32)
        xh = xt.astype(BF)
        xl = (xt - xh.astype(np.float32)).astype(BF)
        return xh, xl

    def run(self, x, w_gate, fc1_w, fc1_b, fc2_w, fc2_b, ncores=NCORES):
        if self.nc is None:
            self.build()
        shared = self.marshal_shared(w_gate, fc1_w, fc1_b, fc2_w, fc2_b)
        in_maps = []
        for c in range(ncores):
            m = dict(shared)
            m["xth"], m["xtl"] = self.marshal_x(
                x[c * self.bsh:(c + 1) * self.bsh])
            in_maps.append(m)
        res = run_bass_kernel_spmd(self.nc, in_maps, core_ids=list(range(ncores)))
        out = np.concatenate(
            [res.results[c]["out"] for c in range(ncores)], axis=1)
        return np.ascontiguousarray(out.astype(np.float32)), res


_KERNEL = None


def kernel(x, w_gate, fc1_w, fc1_b, fc2_w, fc2_b):
    global _KERNEL
    x = np.asarray(x, dtype=np.float32)
    w_gate = np.asarray(w_gate, dtype=np.float32)
    fc1_w = np.asarray(fc1_w, dtype=np.float32)
    fc1_b = np.asarray(fc1_b, dtype=np.float32)
    fc2_w = np.asarray(fc2_w, dtype=np.float32)
    fc2_b = np.asarray(fc2_b, dtype=np.float32)
    if _KERNEL is None:
        _KERNEL = MMoEKernel(use_b2=bool(np.any(fc2_b)))
    out, _ = _KERNEL.run(x, w_gate, fc1_w, fc1_b, fc2_w, fc2_b)
    return out


# revision 20
# speedup vs baseline: 1.2569x; 1.0100x over previous
"""Trainium2 Bass kernel for MMoE (3 tasks, 16 experts, top-4 gating).

Strategy: data-parallel over the batch. Each of the 8 NeuronCores gets
B/8 = 512 tokens and a full (bf16) replica of the expert weights, computes
gating + all 16 expert MLPs + the log-sum-exp combine for its shard, and
writes its [3, 512, 1024] slice. No collectives. Gating logits are computed
in fp32 so top-4 selection matches the reference; expert matmuls run in
bf16 with fp32 PSUM accumulation.

Per-core layout notes:
 - fc1 runs weight-stationary (lhsT = w1^T chunks) so h comes out transposed
   [j, b] — exactly the lhsT layout fc2 needs (contraction over j), avoiding
   any transposes.
 - exp(out) on ScalarE; combine[t] += gate[t,b,e] * exp(out) as a single
   fused scalar_tensor_tensor MAC on VectorE with the gate as a
   per-partition scalar.
 - fc biases are applied for generality: fc1_b via the Relu activation's
   per-partition bias, fc2_b via a K=1 ones-row matmul into PSUM.
"""
import numpy as np
import ml_dtypes

import concourse.mybir as mybir
import concourse.tile as tile
from concourse import bacc
from concourse.bass_utils import run_bass_kernel_spmd

F32 = mybir.dt.float32
BF16 = mybir.dt.bfloat16
AF = mybir.ActivationFunctionType
ALU = mybir.AluOpType
AX = mybir.AxisListType
BF = ml_dtypes.bfloat16

T, B, IN, HID, OUT, E, TOPK = 3, 4096, 1024, 2048, 1024, 16, 4
NCORES = 8
P = 128


class MMoEKernel:
    def __init__(self, bsh=B // NCORES, cin=IN, hid=HID, cout=OUT, ne=E, nt=T,
                 use_b2=True):
        self.bsh, self.cin, self.hid, self.cout, self.ne, self.nt = (
            bsh, cin, hid, cout, ne, nt)
        self.use_b2 = use_b2
        self.nbt = bsh // P
        self.nic = cin // P
        self.njt = hid // P
        self.noh = max(cout // 512, 1)
        self.osz = min(cout, 512)
        self.nq = min(4, self.njt)          # fc1 weight stream granularity
        self.jq = self.njt // self.nq       # j-tiles per fc1 quarter
        self.jh = self.njt // 2             # j-chunks per fc2 half
        self.ng = nt * ne
        self.nc = None

    # ---------------- device graph ----------------
    def build(self):
        bsh, cin, hid, cout, ne, nt = (
            self.bsh, self.cin, self.hid, self.cout, self.ne, self.nt)
        nbt, nic, njt, noh, osz = self.nbt, self.nic, self.njt, self.noh, self.osz
        nq, jq, jh, ng = self.nq, self.jq, self.jh, self.ng

        nc = bacc.Bacc(None, target_bir_lowering=False, debug=False)
        xth = nc.declare_dram_parameter("xth", [P, nic, bsh], BF16, isOutput=False)
        xtl = nc.declare_dram_parameter("xtl", [P, nic, bsh], BF16, isOutput=False)
        wgh = nc.declare_dram_parameter("wgh", [P, nic, ng], BF16, isOutput=False)
        wgl = nc.declare_dram_parameter("wgl", [P, nic, ng], BF16, isOutput=False)
        w1t = nc.declare_dram_parameter(
            "w1t", [ne, nq, P, nic, hid // nq], BF16, isOutput=False)
        w2t = nc.declare_dram_parameter(
            "w2t", [ne, 2, P, jh, cout], BF16, isOutput=False)
        b1t = nc.declare_dram_parameter("b1t", [P, ne * njt], F32, isOutput=False)
        b2 = nc.declare_dram_parameter("b2", [ne, cout], BF16, isOutput=False)
        out_ext = nc.declare_dram_parameter(
            "out", [nt, bsh, cout], F32, isOutput=True)

        with tile.TileContext(nc) as tc:
            import contextlib
            with contextlib.ExitStack() as ctx:
                const = ctx.enter_context(tc.tile_pool(name="const", bufs=1))
                xf_p = ctx.enter_context(tc.tile_pool(name="xf", bufs=1))
                xb_p = ctx.enter_context(tc.tile_pool(name="xb", bufs=1))
                gat_p = ctx.enter_context(tc.tile_pool(name="gat", bufs=1))
                top_p = ctx.enter_context(tc.tile_pool(name="top", bufs=2))
                w1_p = ctx.enter_context(tc.tile_pool(name="w1", bufs=2))
                w2_p = ctx.enter_context(tc.tile_pool(name="w2", bufs=2))
                b2_p = ctx.enter_context(tc.tile_pool(name="b2", bufs=2))
                h_p = ctx.enter_context(tc.tile_pool(name="h", bufs=2))
                eg_p = ctx.enter_context(tc.tile_pool(name="eg", bufs=2))
                comb_p = ctx.enter_context(tc.tile_pool(name="comb", bufs=1))
                pg_p = ctx.enter_context(
                    tc.tile_pool(name="pg", bufs=2, space="PSUM"))
                ph_p = ctx.enter_context(
                    tc.tile_pool(name="ph", bufs=2, space="PSUM"))
                po_p = ctx.enter_context(
                    tc.tile_pool(name="po", bufs=2, space="PSUM"))

                # resident inputs (x in bf16 hi+lo split: hi carries bf16(x),
                # lo the residual, so gating logits reach ~fp32 accuracy with
                # only bf16 matmuls in the PE stream). Critical-path DMAs
                # first: gating needs x+wg, the first fc1 matmul needs w1 q0.
                xbf = xb_p.tile([P, nic, bsh], BF16)
                nc.sync.dma_start(out=xbf[:], in_=xth[:, :, :])
                wg_h = const.tile([P, nic, ng], BF16)
                nc.sync.dma_start(out=wg_h[:], in_=wgh[:, :, :])
                wg_l = const.tile([P, nic, ng], BF16)
                nc.sync.dma_start(out=wg_l[:], in_=wgl[:, :, :])
                xlo = xf_p.tile([P, nic, bsh], BF16)
                nc.sync.dma_start(out=xlo[:], in_=xtl[:, :, :])
                pre_w1sb = w1_p.tile([P, nic, hid // nq], BF16, tag="w1sb")
                nc.sync.dma_start(out=pre_w1sb[:], in_=w1t[0, 0, :, :, :])
                b1sb = const.tile([P, ne * njt], F32)
                nc.sync.dma_start(out=b1sb[:], in_=b1t[:, :])
                pre_w2h = []
                for h in range(2):
                    w2sb = w2_p.tile([P, jh, cout], BF16, tag=f"w2h{h}")
                    nc.sync.dma_start(out=w2sb[:], in_=w2t[0, h, :, :, :])
                    pre_w2h.append(w2sb)
                pre_b2e = b2_p.tile([1, cout], BF16, tag="b2e")
                nc.sync.dma_start(out=pre_b2e[:], in_=b2[0:1, :])
                ones = const.tile([1, P], BF16)
                nc.vector.memset(ones[:], 1.0)
                gates = gat_p.tile([P, nbt, ng], F32)
                comb = comb_p.tile([P, nt * nbt, cout], F32)

                # ---------------- gating (fp32) ----------------
                for bt in range(nbt):
                    pg = pg_p.tile([P, ng], F32)
                    pairs = [(xbf, wg_h), (xbf, wg_l), (xlo, wg_h), (xlo, wg_l)]
                    for pi, (xa, wa) in enumerate(pairs):
                        for ic in range(nic):
                            nc.tensor.matmul(
                                pg[:], lhsT=xa[:, ic, bt * P:(bt + 1) * P],
                                rhs=wa[:, ic, :],
                                start=(pi == 0 and ic == 0),
                                stop=(pi == 3 and ic == nic - 1))
                    gl = top_p.tile([P, ng], F32, tag="gl")
                    nc.scalar.copy(gl[:], pg[:])
                    for t in range(nt):
                        lg = gl[:, t * ne:(t + 1) * ne]
                        m8 = top_p.tile([P, 8], F32, tag="m8")
                        nc.vector.max(m8[:], lg)
                        negm1 = top_p.tile([P, 1], F32, tag="negm1")
                        nc.vector.tensor_scalar_mul(negm1[:], m8[:, 0:1], -1.0)
                        s = top_p.tile([P, ne], F32, tag="s")
                        nc.scalar.activation(s[:], lg, AF.Exp, bias=negm1[:])
                        ind = top_p.tile([P, ne], F32, tag="ind")
                        nc.vector.tensor_scalar(
                            ind[:], lg, m8[:, TOPK - 1:TOPK], None, op0=ALU.is_ge)
                        gun = top_p.tile([P, ne], F32, tag="gun")
                        nc.vector.tensor_mul(gun[:], s[:], ind[:])
                        z = top_p.tile([P, 1], F32, tag="z")
                        nc.vector.reduce_sum(z[:], gun[:], axis=AX.X)
                        rz = top_p.tile([P, 1], F32, tag="rz")
                        nc.vector.reciprocal(rz[:], z[:])
                        gg = top_p.tile([P, ne], F32, tag="gg")
                        nc.vector.tensor_scalar_mul(gg[:], gun[:], rz[:])
                        keep = top_p.tile([P, ne], F32, tag="keep")
                        nc.vector.tensor_scalar(
                            keep[:], gg[:], 1e-4, None, op0=ALU.is_gt)
                        nc.vector.tensor_mul(
                            gates[:, bt, t * ne:(t + 1) * ne], gg[:], keep[:])

                # ---------------- expert loop ----------------
                for e in range(ne):
                    if e == 0:
                        w2h = pre_w2h
                        b2e = pre_b2e
                    else:
                        w2h = []
                        for h in range(2):
                            w2sb = w2_p.tile([P, jh, cout], BF16, tag=f"w2h{h}")
                            nc.sync.dma_start(
                                out=w2sb[:], in_=w2t[e, h, :, :, :])
                            w2h.append(w2sb)
                        b2e = b2_p.tile([1, cout], BF16, tag="b2e")
                        nc.sync.dma_start(out=b2e[:], in_=b2[e:e + 1, :])
                    hT = h_p.tile([P, njt, bsh], BF16, tag="hT")
                    w1sb = None
                    for jt in range(njt):
                        q, jj = divmod(jt, jq)
                        if jj == 0:
                            if e == 0 and q == 0:
                                w1sb = pre_w1sb
                            else:
                                w1sb = w1_p.tile(
                                    [P, nic, hid // nq], BF16, tag="w1sb")
                                nc.sync.dma_start(
                                    out=w1sb[:], in_=w1t[e, q, :, :, :])
                        ph = ph_p.tile([P, bsh], F32)
                        for ic in range(nic):
                            nc.tensor.matmul(
                                ph[:], lhsT=w1sb[:, ic, jj * P:(jj + 1) * P],
                                rhs=xbf[:, ic, :],
                                start=(ic == 0), stop=(ic == nic - 1))
                        nc.scalar.activation(
                            hT[:, jt, :], ph[:], AF.Relu,
                            bias=b1sb[:, e * njt + jt: e * njt + jt + 1])
                    for bt in range(nbt):
                        po = po_p.tile([P, cout], F32)
                        if self.use_b2:
                            for oh in range(noh):
                                nc.tensor.matmul(
                                    po[:, oh * osz:(oh + 1) * osz],
                                    lhsT=ones[:, :],
                                    rhs=b2e[:, oh * osz:(oh + 1) * osz],
                                    start=True, stop=False)
                        for jc in range(njt):
                            hh, jj = divmod(jc, jh)
                            for oh in range(noh):
                                nc.tensor.matmul(
                                    po[:, oh * osz:(oh + 1) * osz],
                                    lhsT=hT[:, jc, bt * P:(bt + 1) * P],
                                    rhs=w2h[hh][:, jj, oh * osz:(oh + 1) * osz],
                                    start=(jc == 0 and not self.use_b2),
                                    stop=(jc == njt - 1))
                        eg = eg_p.tile([P, cout], F32)
                        nc.scalar.activation(eg[:], po[:], AF.Exp)
                        for t in range(nt):
                            gcol = gates[:, bt, t * ne + e: t * ne + e + 1]
                            dst = comb[:, t * nbt + bt, :]
                            if e == 0:
                                nc.vector.tensor_scalar_mul(dst, eg[:], gcol)
                            else:
                                nc.vector.scalar_tensor_tensor(
                                    dst, eg[:], gcol, dst,
                                    op0=ALU.mult, op1=ALU.add)

                # ---------------- log + output ----------------
                for t in range(nt):
                    for bt in range(nbt):
                        cslice = comb[:, t * nbt + bt, :]
                        nc.scalar.activation(cslice, cslice, AF.Ln)
                        nc.sync.dma_start(
                            out=out_ext[t, bt * P:(bt + 1) * P, :], in_=cslice)

        nc.compile()
        self.nc = nc
        return nc

    # ---------------- host-side marshalling ----------------
    def marshal_shared(self, w_gate, fc1_w, fc1_b, fc2_w, fc2_b):
        cin, hid, cout, ne, nt = self.cin, self.hid, self.cout, self.ne, self.nt
        nic, njt, nq, jh, ng = self.nic, self.njt, self.nq, self.jh, self.ng
        wgt = np.ascontiguousarray(
            w_gate.transpose(1, 0, 2).reshape(cin, ng)
            .reshape(nic, P, ng).transpose(1, 0, 2)).astype(np.float32)
        wgh = wgt.astype(BF)
        wgl = (wgt - wgh.astype(np.float32)).astype(BF)
        w1t = np.empty((ne, nq, P, nic, hid // nq), dtype=BF)
        w2t = np.empty((ne, 2, P, jh, cout), dtype=BF)
        for e in range(ne):
            a = fc1_w[e].T.reshape(nic, P, hid).transpose(1, 0, 2)
            for q in range(nq):
                w1t[e, q] = a[:, :, q * (hid // nq):(q + 1) * (hid // nq)]
            bm = fc2_w[e].T.reshape(njt, P, cout).transpose(1, 0, 2)
            for h in range(2):
                w2t[e, h] = bm[:, h * jh:(h + 1) * jh, :]
        b1t = np.ascontiguousarray(
            fc1_b.reshape(ne, njt, P).transpose(2, 0, 1)
            .reshape(P, ne * njt)).astype(np.float32)
        b2m = np.ascontiguousarray(fc2_b).astype(BF)
        return dict(wgh=wgh, wgl=wgl, w1t=w1t, w2t=w2t, b1t=b1t, b2=b2m)

    def marshal_x(self, x_shard):
        xt = np.ascontiguousarray(
            x_shard.T.reshape(self.nic, P, self.bsh).transpose(1, 0, 2)
        ).astype(np.float32)
        xh = xt.astype(BF)
        xl = (xt - xh.astype(np.float32)).astype(BF)
        return xh, xl

    def run(self, x, w_gate, fc1_w, fc1_b, fc2_w, fc2_b, ncores=NCORES):
        if self.nc is None:
            self.build()
        shared = self.marshal_shared(w_gate, fc1_w, fc1_b, fc2_w, fc2_b)
        in_maps = []
        for c in range(ncores):
            m = dict(shared)
            m["xth"], m["xtl"] = self.marshal_x(
                x[c * self.bsh:(c + 1) * self.bsh])
            in_maps.append(m)
        res = run_bass_kernel_spmd(self.nc, in_maps, core_ids=list(range(ncores)))
        out = np.concatenate(
            [res.results[c]["out"] for c in range(ncores)], axis=1)
        return np.ascontiguousarray(out.astype(np.float32)), res


_KERNEL = None


def kernel(x, w_gate, fc1_w, fc1_b, fc2_w, fc2_b):
    global _KERNEL
    x = np.asarray(x, dtype=np.float32)
    w_gate = np.asarray(w_gate, dtype=np.float32)
    fc1_w = np.asarray(fc1_w, dtype=np.float32)
    fc1_b = np.asarray(fc1_b, dtype=np.float32)
    fc2_w = np.asarray(fc2_w, dtype=np.float32)
    fc2_b = np.asarray(fc2_b, dtype=np.float32)
    if _KERNEL is None:
        _KERNEL = MMoEKernel(use_b2=bool(np.any(fc2_b)))
    out, _ = _KERNEL.run(x, w_gate, fc1_w, fc1_b, fc2_w, fc2_b)
    return out
